# revision 10
# baseline (speedup 1.0000x reference)
"""Trainium2 Bass kernel for nn_Attention_82815559401482 (sparse_attention).

Full-input contract: kernel(**inputs) takes the complete (unsharded) inputs
and returns the full [16, 784, 512] output. Internally shards data-parallel
over the batch dim across 8 NeuronCores (2 batches per core), builds one SPMD
Bass/Tile program, and runs it via run_bass_kernel_spmd.

Math (per core, b in {0,1} local batches):
  qkv = BN(x @ w_qkv^T)           -> folded into w/b on host, q pre-scaled
  S^T[key,q] = k·q + bias         -> bias applied multiplicatively post-exp:
  E = exp(S^T_raw) * exp(bias)    (exp(bias) precomputed on host, bf16)
  U = V^T-weighted sums: U[d,q] = sum_k v[k,d] E[k,q]   (bf16 matmul)
  Z[q] = sum_k E[k,q]             (ones-matmul, output replicated over 128 p)
  O^T = U/Z + bv ; hardswish ; proj with folded BN (+ /6 folded into w_proj)
"""

import os
import sys

import numpy as np


def _ensure_deps():
    try:
        import concourse.bass  # noqa: F401
        return
    except ImportError:
        pass
    for p in ("/opt/trn_rl_repo", "/root/.axon_site/_ro/trn_rl_repo"):
        if os.path.isdir(p) and p not in sys.path:
            sys.path.insert(0, p)
    import concourse.bass  # noqa: F401


_ensure_deps()

import ml_dtypes  # noqa: E402
import concourse.bass as bass  # noqa: E402
import concourse.mybir as mybir  # noqa: E402
import concourse.tile as tile  # noqa: E402
from concourse.alu_op_type import AluOpType  # noqa: E402
from concourse.vector_clock import ScopedClock  # noqa: E402
from concourse.bass_utils import run_bass_kernel_spmd  # noqa: E402
from contextlib import ExitStack  # noqa: E402


def _patch_tile_drain():
    """The installed walrus rejects >1 semaphore wait on one SP CTRL
    instruction ("Too many sync wait commands"); TileContext's tail drain
    puts one wait per live semaphore on a single Drain. Split the waits
    across dedicated nop instructions instead."""
    if getattr(tile.TileContext, "_drain_patched", False):
        return

    def _drain_and_barrier(self, tick_clock, wait_clock):
        nc = self.nc
        drain_inst = nc.sync.drain()
        wait_clock.add_sem_waits(
            drain_inst.ins, ScopedClock({None: tick_clock.global_clock})
        )
        si = drain_inst.ins.sync_info
        waits = list(si.on_wait or [])
        if len(waits) > 1:
            si.on_wait.clear()
            for w in waits:
                w_inst = nc.sync.nop(nofuse=True, hint="drain_wait")
                w_inst.ins.sync_info = mybir.SyncInfo(on_wait=[w], on_update=[])
        nc.all_engine_barrier()
        assert self.sems is not None
        popped = nc._tile_sem_poison_stack.pop()
        assert popped is self._sem_poison
        nc.clear_and_free_semaphores(list(self.sems.allocated().values()))
        nc.all_engine_barrier()

    tile.TileContext._drain_and_barrier = _drain_and_barrier
    tile.TileContext._drain_patched = True


_patch_tile_drain()


def _split_multi_waits(nc):
    """This walrus build rejects instructions carrying more than one
    semaphore wait ("Too many sync wait commands"). Hoist extra waits onto
    same-engine nop instructions inserted just before the instruction."""
    n = 0
    for fn in nc.m.functions:
        for blk in fn.blocks:
            new_insts = []
            for inst in blk.instructions:
                si = inst.sync_info
                if si is not None and si.on_wait and len(si.on_wait) > 1:
                    waits = list(si.on_wait)
                    for i, w in enumerate(waits[1:]):
                        nop = mybir.InstNoOp(
                            name=f"{inst.name}_xw{i}",
                            engine=inst.engine,
                            bass_nofuse=True,
                            sync_info=mybir.SyncInfo(on_wait=[w], on_update=[]),
                        )
                        new_insts.append(nop)
                        n += 1
                    si.on_wait.clear()
                    si.on_wait.append(waits[0])
                new_insts.append(inst)
            blk.instructions.clear()
            blk.instructions.extend(new_insts)
    return n


# Problem dims (hardcoded per contract)
B, RES, DIM = 16, 28, 512
N = RES * RES  # 784
H, KD = 8, 32
D = 128  # v head dim
DH = D * H  # 1024
EPS = 1e-5
SCALE = KD ** -0.5

NCORES = 8
BPC = B // NCORES  # 2 batches per core
T = BPC * N  # 1568 tokens per core

FP = mybir.dt.float32
FR = mybir.dt.float32r
BF = mybir.dt.bfloat16
FH = mybir.dt.float16

KCH = [(i * 128, min(128, N - i * 128)) for i in range((N + 127) // 128)]  # 7 chunks
QBL = [(0, 512), (512, 272)]  # query free-dim blocks within 784
TBL = [(o, min(512, T - o)) for o in range(0, T, 512)]  # token blocks of 1568
DIMC = DIM // 128  # 4
DHC = DH // 128  # 8

AFT = mybir.ActivationFunctionType

_PROGRAM_CACHE = {}


def build_program():
    nc = bass.Bass("TRN2", target_bir_lowering=False, debug=False,
                   num_devices=NCORES)

    xT = nc.dram_tensor("xT", [DIM, T], FH, kind="ExternalInput").ap()
    wqkT = nc.dram_tensor("wqkT", [DIM, 512], FH, kind="ExternalInput").ap()
    wvT = nc.dram_tensor("wvT", [DIM, DH], FH, kind="ExternalInput").ap()
    wpT = nc.dram_tensor("wpT", [DH, DIM], FH, kind="ExternalInput").ap()
    bqk = nc.dram_tensor("bqk", [512], FP, kind="ExternalInput").ap()
    bvrow = nc.dram_tensor("bvrow", [128, DH], FP, kind="ExternalInput").ap()
    bp = nc.dram_tensor("bp", [DIM], FP, kind="ExternalInput").ap()
    eb = nc.dram_tensor("eb", [H, 7 * 128, N], FH, kind="ExternalInput").ap()
    out = nc.dram_tensor("out", [DIM, T], FP, kind="ExternalOutput").ap()

    with tile.TileContext(nc) as tc, ExitStack() as ctx:
        # ---------- persistent pools ----------
        wpool = ctx.enter_context(tc.tile_pool(name="w", bufs=1))
        qkpool = ctx.enter_context(tc.tile_pool(name="qk", bufs=1))
        cpool = ctx.enter_context(tc.tile_pool(name="consts", bufs=1))
        dram = ctx.enter_context(tc.tile_pool(name="dram", bufs=1, space="DRAM"))

        # weights, packed: wqk_sb free dim holds the 4 dim-chunks of [128,512]
        wqk_sb = wpool.tile([128, DIMC * 512], FH, tag="wqk")
        wv_sb = wpool.tile([128, DIMC * DH], FH, tag="wv")
        wp_sb = wpool.tile([128, DHC * 512], FH, tag="wp")
        for c in range(DIMC):
            nc.sync.dma_start(wqk_sb[:, c * 512:(c + 1) * 512],
                              wqkT[c * 128:(c + 1) * 128, :])
            nc.sync.dma_start(wv_sb[:, c * DH:(c + 1) * DH],
                              wvT[c * 128:(c + 1) * 128, :])
        for c in range(DHC):
            nc.sync.dma_start(wp_sb[:, c * 512:(c + 1) * 512],
                              wpT[c * 128:(c + 1) * 128, :])

        bqk_sb = cpool.tile([128, DIMC], FP, tag="bqk")
        nc.sync.dma_start(bqk_sb[:, :], bqk.rearrange("(c p) -> p c", p=128))
        bvrow_sb = cpool.tile([128, DH], FP, tag="bvrow")
        nc.sync.dma_start(bvrow_sb[:, :], bvrow[:, :])
        bp_sb = cpool.tile([128, DIMC], FP, tag="bp")
        nc.sync.dma_start(bp_sb[:, :], bp.rearrange("(c p) -> p c", p=128))
        ones_sb = cpool.tile([128, 128], FH, tag="ones")
        nc.vector.memset(ones_sb[:, :], 1.0)
        three_sb = cpool.tile([128, 1], FP, tag="three")
        nc.vector.memset(three_sb[:, :], 3.0)

        # qk^T activations [512 ch, T]: 4 m-chunks at free offsets.
        # m-chunk 0: q heads 0-3 (32 rows each), 1: q heads 4-7,
        # 2: k heads 0-3, 3: k heads 4-7.
        qkT_sb = qkpool.tile([128, 4 * T], FH, tag="qkT")

        # DRAM scratch
        NPAD = 7 * 128  # 896, padded batch-local token count
        vscr = dram.tile([BPC * NPAD, DH], FH, tag="vscr")
        oscr = dram.tile([DH, T], FH, tag="oscr")

        # ---------- stage 1: qkv projection ----------
        with tc.tile_pool(name="s1", bufs=2) as s1pool, \
             tc.tile_pool(name="ps1", bufs=1, space="PSUM") as ps1:
            xT_sb = s1pool.tile([128, DIMC * T], FH, tag="xT", bufs=1)
            for c in range(DIMC):
                nc.sync.dma_start(xT_sb[:, c * T:(c + 1) * T],
                                  xT[c * 128:(c + 1) * 128, :])

            # q/k: out [128 ch, token-block]
            for mc in range(4):
                for (no, nn) in TBL:
                    ps = ps1.tile([128, 512], FP, tag="qkps", bufs=2)
                    for c in range(DIMC):
                        nc.tensor.matmul(
                            ps[:, :nn],
                            lhsT=wqk_sb[:, c * 512 + mc * 128:
                                        c * 512 + (mc + 1) * 128],
                            rhs=xT_sb[:, c * T + no:c * T + no + nn],
                            start=(c == 0), stop=(c == DIMC - 1))
                    nc.scalar.activation(qkT_sb[:, mc * T + no:mc * T + no + nn],
                                         ps[:, :nn], AFT.Identity,
                                         bias=bqk_sb[:, mc:mc + 1])

            # v: out [token-chunk, v-channel-block] -> bf16 -> DRAM scratch
            for b in range(BPC):
                for (ko, kn) in KCH:
                    to = b * N + ko
                    tp = b * NPAD + ko
                    for nb in range(2):
                        ps = ps1.tile([128, 512], FP, tag="vps", bufs=2)
                        for c in range(DIMC):
                            nc.tensor.matmul(
                                ps[:kn, :],
                                lhsT=xT_sb[:, c * T + to:c * T + to + kn],
                                rhs=wv_sb[:, c * DH + nb * 512:
                                          c * DH + (nb + 1) * 512],
                                start=(c == 0), stop=(c == DIMC - 1))
                        vst = s1pool.tile([128, 512], FH, tag="vst", bufs=3)
                        nc.vector.tensor_tensor(
                            vst[:kn, :], ps[:kn, :],
                            bvrow_sb[:kn, nb * 512:(nb + 1) * 512],
                            op=AluOpType.add)
                        nc.sync.dma_start(
                            vscr[tp:tp + kn, nb * 512:(nb + 1) * 512],
                            vst[:kn, :])

        # ---------- stage 2: attention ----------
        with tc.tile_pool(name="s2", bufs=2) as s2pool, \
             tc.tile_pool(name="ps2", bufs=1, space="PSUM") as ps2:
            for h in range(H):
                hq = (h // 4) * T          # q m-chunk free offset
                hk = (2 + h // 4) * T      # k m-chunk free offset
                hp = 32 * (h % 4)          # partition base within m-chunk

                eb_t = s2pool.tile([128, 7 * N], FH, tag="ebias", bufs=2)
                nc.sync.dma_start(
                    eb_t[:, :],
                    eb[h].rearrange("(c p) q -> p (c q)", p=128))

                for b in range(BPC):
                    to = b * N
                    u_ps = ps2.tile([128, N], FP, tag="u", bufs=1)
                    z_ps = ps2.tile([128, N], FP, tag="z", bufs=1)
                    v_t = s2pool.tile([128, 7 * 128], FH, tag="vt", bufs=2)
                    nc.sync.dma_start(
                        v_t[:, :],
                        vscr[b * NPAD:(b + 1) * NPAD,
                             h * 128:(h + 1) * 128
                             ].rearrange("(c p) d -> p (c d)", p=128))
                    for i, (ko, kn) in enumerate(KCH):
                        s_ps = ps2.tile([128, N], FP, tag="s", bufs=2)
                        for (qo, qn) in QBL:
                            nc.tensor.matmul(
                                s_ps[:kn, qo:qo + qn],
                                lhsT=qkT_sb[hp:hp + 32,
                                            hk + to + ko:hk + to + ko + kn
                                            ],
                                rhs=qkT_sb[hp:hp + 32,
                                           hq + to + qo:hq + to + qo + qn
                                           ],
                                start=True, stop=True,
                                tile_position=(hp, 0))
                        e_t = s2pool.tile([128, N], FH, tag="e", bufs=4)
                        nc.scalar.activation(e_t[:kn, :], s_ps[:kn, :], AFT.Exp)
                        e2_t = s2pool.tile([128, N], FH, tag="e2", bufs=7)
                        nc.vector.tensor_tensor(
                            e2_t[:kn, :], e_t[:kn, :],
                            eb_t[:kn, i * N:(i + 1) * N], op=AluOpType.mult)
                        for (qo, qn) in QBL:
                            nc.tensor.matmul(u_ps[:, qo:qo + qn],
                                             lhsT=v_t[:kn,
                                                      i * 128:i * 128 + 128],
                                             rhs=e2_t[:kn, qo:qo + qn],
                                             start=(i == 0), stop=(i == 6))
                            nc.tensor.matmul(z_ps[:, qo:qo + qn],
                                             lhsT=ones_sb[:kn, :],
                                             rhs=e2_t[:kn, qo:qo + qn],
                                             start=(i == 0), stop=(i == 6))

                    # normalize + bias + hardswish (x*relu6(x+3); /6 folded
                    # into w_proj on host)
                    r_t = s2pool.tile([128, N], FP, tag="r", bufs=2)
                    nc.vector.reciprocal_approx_fast(r_t[:, :], z_ps[:, :])
                    d_t = s2pool.tile([128, N], FP, tag="d", bufs=2)
                    nc.vector.tensor_tensor(d_t[:, :], u_ps[:, :], r_t[:, :],
                                            op=AluOpType.mult)
                    a_t = s2pool.tile([128, N], FP, tag="a", bufs=2)
                    nc.scalar.activation(a_t[:, :], d_t[:, :], AFT.Relu,
                                         bias=three_sb[:, 0:1])
                    o_t = s2pool.tile([128, N], FH, tag="ost", bufs=3)
                    nc.vector.scalar_tensor_tensor(o_t[:, :], a_t[:, :], 6.0,
                                                   d_t[:, :],
                                                   op0=AluOpType.min,
                                                   op1=AluOpType.mult)
                    nc.sync.dma_start(oscr[h * 128:(h + 1) * 128, to:to + N],
                                      o_t[:, :])

        # ---------- stage 3: output projection ----------
        with tc.tile_pool(name="s3", bufs=2) as s3pool, \
             tc.tile_pool(name="ps3", bufs=1, space="PSUM") as ps3:
            for (no, nn) in TBL:
                pj = [ps3.tile([128, 512], FP, tag=f"pj{c4}", bufs=1,
                               name=f"pj{c4}_{no}")
                      for c4 in range(DIMC)]
                for dhc in range(DHC):
                    o_in = s3pool.tile([128, 512], FH, tag="oin", bufs=4)
                    nc.sync.dma_start(o_in[:, :nn],
                                      oscr[dhc * 128:(dhc + 1) * 128,
                                           no:no + nn])
                    for c4 in range(DIMC):
                        nc.tensor.matmul(
                            pj[c4][:, :nn],
                            lhsT=wp_sb[:, dhc * 512 + c4 * 128:
                                       dhc * 512 + (c4 + 1) * 128],
                            rhs=o_in[:, :nn],
                            start=(dhc == 0), stop=(dhc == DHC - 1))
                for c4 in range(DIMC):
                    o_st = s3pool.tile([128, 512], FP, tag="outst", bufs=4)
                    nc.scalar.activation(o_st[:, :nn], pj[c4][:, :nn],
                                         AFT.Identity,
                                         bias=bp_sb[:, c4:c4 + 1])
                    nc.sync.dma_start(out[c4 * 128:(c4 + 1) * 128, no:no + nn],
                                      o_st[:, :nn])

    # populate .instr bytes for InstISA (custom-DVE ops) — raw Bass skips this
    mybir.codegen_inst_isa_subclasses(nc)
    nsplit = _split_multi_waits(nc)
    if os.environ.get("KERNEL_DEBUG"):
        print(f"[kernel] split {nsplit} multi-wait instructions")
    return nc


def _prepare_host_inputs(x, w_qkv, qkv_g, qkv_b, qkv_m, qkv_v, ab, w_proj,
                         proj_g, proj_b, proj_m, proj_v, bias_idx):
    f32 = np.float32
    x = np.asarray(x, f32)
    w_qkv = np.asarray(w_qkv, f32)
    qkv_g = np.asarray(qkv_g, f32)
    qkv_b = np.asarray(qkv_b, f32)
    qkv_m = np.asarray(qkv_m, f32)
    qkv_v = np.asarray(qkv_v, f32)
    ab = np.asarray(ab, f32)
    w_proj = np.asarray(w_proj, f32)
    proj_g = np.asarray(proj_g, f32)
    proj_b = np.asarray(proj_b, f32)
    proj_m = np.asarray(proj_m, f32)
    proj_v = np.asarray(proj_v, f32)
    bias_idx = np.asarray(bias_idx)

    # fold qkv BN: y = (x@W^T)*s + (b - m*s)
    s = qkv_g / np.sqrt(qkv_v + EPS)
    w_f = w_qkv * s[:, None]
    b_f = qkv_b - qkv_m * s

    # channel c = h*192 + i; i<32 q (pre-scale by SCALE), <64 k, else v
    q_rows = [w_f[h * 192:h * 192 + 32] * SCALE for h in range(H)]
    k_rows = [w_f[h * 192 + 32:h * 192 + 64] for h in range(H)]
    v_rows = [w_f[h * 192 + 64:h * 192 + 192] for h in range(H)]
    q_b = [b_f[h * 192:h * 192 + 32] * SCALE for h in range(H)]
    k_b = [b_f[h * 192 + 32:h * 192 + 64] for h in range(H)]
    v_b = [b_f[h * 192 + 64:h * 192 + 192] for h in range(H)]

    w_qk = np.concatenate(q_rows + k_rows, axis=0)      # [512, 512]
    bqk = np.concatenate(q_b + k_b, axis=0)             # [512]
    w_v = np.concatenate(v_rows, axis=0)                # [1024, 512]
    bv = np.concatenate(v_b, axis=0)                    # [1024]

    wqkT = np.ascontiguousarray(w_qk.T)                 # [512 dim, 512 ch]
    wvT = np.ascontiguousarray(w_v.T)                   # [512, 1024]

    # fold proj BN + hardswish /6: P = hs6(o) @ (W*s/6)^T + (b - m*s)
    sp = proj_g / np.sqrt(proj_v + EPS)
    w_p = w_proj * sp[:, None] / 6.0
    bpv = proj_b - proj_m * sp
    wpT = np.ascontiguousarray(w_p.T)                   # [1024, 512]

    # multiplicative positional bias: exp(ab[h, bias_idx]) in bf16
    ebias = np.zeros((H, 7 * 128, N), np.float16)  # padded to 896 key rows
    ebias[:, :N, :] = np.exp(ab[:, bias_idx]).astype(np.float16)

    wqkT = wqkT.astype(np.float16)
    wvT = wvT.astype(np.float16)
    wpT = wpT.astype(np.float16)

    # x transposed: [B, DIM, N] then per-core concat of its 2 batches
    xT_all = np.ascontiguousarray(x.transpose(0, 2, 1).astype(np.float16))

    in_maps = []
    for c in range(NCORES):
        xt_core = np.ascontiguousarray(
            np.concatenate([xT_all[BPC * c + b] for b in range(BPC)], axis=1))
        in_maps.append(dict(
            xT=xt_core, wqkT=wqkT, wvT=wvT, wpT=wpT,
            bqk=np.ascontiguousarray(bqk),
            bvrow=np.ascontiguousarray(np.broadcast_to(bv, (128, DH))),
            bp=np.ascontiguousarray(bpv), eb=ebias,
        ))
    return in_maps


def _get_program():
    if "nc" not in _PROGRAM_CACHE:
        _PROGRAM_CACHE["nc"] = build_program()
    return _PROGRAM_CACHE["nc"]


def run(inputs: dict, trace: bool = False, trace_kwargs: dict | None = None):
    """Build+run; returns (full_output [16,784,512], BassKernelResults)."""
    nc = _get_program()
    in_maps = _prepare_host_inputs(**inputs)
    kw = {}
    if trace:
        kw = dict(trace=True, trace_cores=[0], **(trace_kwargs or {}))
    res = run_bass_kernel_spmd(nc, in_maps, core_ids=list(range(NCORES)), **kw)
    outs = []
    for c in range(NCORES):
        o = res.results[c]["out"]  # [512, 1568]
        o = o.reshape(DIM, BPC, N).transpose(1, 2, 0)  # [2, 784, 512]
        outs.append(o)
    full = np.concatenate(outs, axis=0).astype(np.float32)
    return full, res


def kernel(**inputs) -> np.ndarray:
    out, _ = run(inputs, trace=False)
    return out
